# revision 11
# baseline (speedup 1.0000x reference)
"""MOLELinear (mixture-of-linear-experts) Trainium2 kernel.

Math (per group g): out_g = x_g @ (sum_e c[g,e] W_e + W_sh).T + (sum_e c[g,e] b_e + b_sh)

Sharding: data-parallel over the 32 groups -> 4 groups (8192 tokens) per core,
expert weights replicated. Host does layout-only prep (transpose / stacking /
fp16 rounding); all arithmetic of the reference runs on device.

Device plan per core (all fp16 data path, fp32 accumulation in PSUM):
  - DMA in: xT [512, 8192] fp16 (x shard transposed), wall [128, 9, 2048] fp16
    (shared + 8 experts, k-tile-major free layout), small coefficient/bias
    tensors, cdiag [128, 9, 128] (scaled identity matrices for group 0).
  - Group-0 weight mix on PE during the weight-DMA window:
    psum[:, kt] += diag(c_j) @ W_j[kt]  (keeps PE warm, fp32 accumulation).
  - Groups 1-3 weight mix on DVE: fp16 scalar_tensor_tensor FMA chains,
    FD=2048 per op (8 ops per group).
  - Mixed biases transposed on PE: mbT[ot][o,g] = sum_j ball[j,o] cx[j,g].
  - Main GEMM: stationary = mixed weight subtile [128k,128o], moving = xT
    slice [128k,512t]; psum [128 o, 2048 t] accumulates 4 k-tiles.
  - Drain on ScalarE via ACTIVATE(Identity, bias=mbT column): PSUM->SBUF fp16
    with the per-partition bias add fused in. DMA out on GpSimd (SWDGE) so
    output stores don't contend with the input DMA issue queue.
"""
import numpy as np

import concourse.bacc as bacc
import concourse.mybir as mybir
from concourse.alu_op_type import AluOpType
from concourse.tile import TileContext
from concourse.bass_utils import run_bass_kernel_spmd

N_CORES = 8
IN_F = 512
OUT_F = 512
N_EXPERTS = 8
N_GROUPS = 32
TOK_PER_GROUP = 2048
G_PER_CORE = N_GROUPS // N_CORES           # 4
TOK_PER_CORE = G_PER_CORE * TOK_PER_GROUP  # 8192
KT = IN_F // 128                           # 4 k-tiles
OT = OUT_F // 128                          # 4 out-feature tiles
NW = N_EXPERTS + 1                         # 9: shared weight first, then experts
F32 = mybir.dt.float32
F32R = mybir.dt.float32r
F16 = mybir.dt.float16
AF = mybir.ActivationFunctionType

_CACHE = {}


def _build():
    nc = bacc.Bacc(trn_type="TRN2")
    xT = nc.dram_tensor("xT", (IN_F, TOK_PER_CORE), F16, kind="ExternalInput")
    wall = nc.dram_tensor("wall", (128, NW, KT * OUT_F), F16, kind="ExternalInput")
    cdiag = nc.dram_tensor("cdiag", (128, NW, 128), F16, kind="ExternalInput")
    cb = nc.dram_tensor("cb", (128, G_PER_CORE * N_EXPERTS), F32, kind="ExternalInput")
    cx = nc.dram_tensor("cx", (NW, G_PER_CORE), F32R, kind="ExternalInput")
    ball = nc.dram_tensor("ball", (NW, OUT_F), F32R, kind="ExternalInput")
    outT = nc.dram_tensor("outT", (OUT_F, TOK_PER_CORE), F16, kind="ExternalOutput")

    with TileContext(nc) as tc:
        with (
            tc.tile_pool(name="smallp", bufs=1) as smallp,
            tc.tile_pool(name="wallp", bufs=1) as wallp,
            tc.tile_pool(name="wmp", bufs=1) as wmp,
            tc.tile_pool(name="xp", bufs=1) as xp,
            tc.tile_pool(name="ocp", bufs=4) as ocp,
        ):
            # ---- small DMAs first (bias inputs before everything) ----
            cxt = smallp.tile([NW, G_PER_CORE], F32R, tag="cx")
            nc.sync.dma_start(cxt[:], cx[:])
            ballt = smallp.tile([NW, OUT_F], F32R, tag="ball")
            nc.sync.dma_start(ballt[:], ball[:])
            cdt = smallp.tile([128, NW * 128], F16, tag="cdt")
            nc.sync.dma_start(cdt[:].rearrange("p (e m) -> p e m", e=NW), cdiag[:])

            # ---- expert weights: one tile+DMA per expert slice (shared j=0) ----
            # separate tiles so slice readers never wait on later experts' DMAs
            wallst = [
                wallp.tile([128, KT * OUT_F], F16, tag=f"wall{j}", name=f"wall{j}")
                for j in range(NW)
            ]
            cbt = smallp.tile([128, G_PER_CORE * N_EXPERTS], F32, tag="cb")
            for j in range(NW):
                nc.sync.dma_start(wallst[j][:], wall[:, j, :])
                if j == 1:
                    # mixing chains need the coefficient broadcast early
                    nc.sync.dma_start(cbt[:], cb[:])

            # ---- x: group 0 in token-chunk-major tiles (first GEMM needs only
            # chunk 0 = 0.5MB), groups 1-3 one whole-group DMA each ----
            xg0t = [
                xp.tile([128, KT * 512], F16, tag=f"x0c{tci}", name=f"x0c{tci}")
                for tci in range(TOK_PER_GROUP // 512)
            ]
            for tci in range(TOK_PER_GROUP // 512):
                nc.sync.dma_start(
                    xg0t[tci][:].rearrange("p (kt t) -> p kt t", kt=KT),
                    xT[:, tci * 512 : (tci + 1) * 512].rearrange(
                        "(kt p) t -> p kt t", p=128
                    ),
                )
            xg = [None]
            for g in range(1, G_PER_CORE):
                t = xp.tile([128, KT * TOK_PER_GROUP], F16, tag=f"x{g}", name=f"x{g}")
                nc.sync.dma_start(
                    t[:].rearrange("p (kt t) -> p kt t", kt=KT),
                    xT[:, g * TOK_PER_GROUP : (g + 1) * TOK_PER_GROUP].rearrange(
                        "(kt p) t -> p kt t", p=128
                    ),
                )
                xg.append(t)

            def xslice(g, kt, tci):
                if g == 0:
                    return xg0t[tci][:, kt * 512 : (kt + 1) * 512]
                return xg[g][
                    :,
                    kt * TOK_PER_GROUP + tci * 512 : kt * TOK_PER_GROUP + (tci + 1) * 512,
                ]

            wm = [
                wmp.tile([128, KT * OUT_F], F16, tag=f"wm{g}", name=f"wm{g}")
                for g in range(G_PER_CORE)
            ]

            with tc.tile_pool(name="ps", bufs=2, space="PSUM") as ps:
                # ---- mixed biases (one bank, freed early):
                # mbT2[o', ot*4+g] = sum_j ball[j, ot*128+o'] cx[j, g]
                pb = ps.tile([128, OT * G_PER_CORE], F32, tag="ps")
                for ot in range(OT):
                    nc.tensor.matmul(
                        pb[:, ot * G_PER_CORE : (ot + 1) * G_PER_CORE],
                        ballt[:, ot * 128 : (ot + 1) * 128],
                        cxt[:],
                        start=True,
                        stop=True,
                    )
                mbT2 = smallp.tile([128, OT * G_PER_CORE], F32, tag="mbT2")
                nc.scalar.copy(mbT2[:], pb[:])

                # ---- groups 1-3 weight mix on DVE, two-step per term:
                # tensor_scalar (4x mode) then tensor_tensor (2x mode)
                for g in (1, 2, 3):
                    for e in range(N_EXPERTS):
                        tmp = wmp.tile(
                            [128, KT * OUT_F], F16, tag="tmp", name="tmp", bufs=2
                        )
                        nc.vector.tensor_scalar(
                            tmp[:],
                            wallst[e + 1][:],
                            cbt[:, g * N_EXPERTS + e : g * N_EXPERTS + e + 1],
                            None,
                            AluOpType.mult,
                        )
                        nc.vector.tensor_tensor(
                            wm[g][:],
                            tmp[:],
                            wallst[0][:] if e == 0 else wm[g][:],
                            AluOpType.add,
                        )

                # ---- group-0 weight mix on PE (runs while weights stream in);
                # last expert's matmuls interleaved with per-k-tile casts so
                # wm[0] is ready ~1.5us after the final weight slice lands
                pm = ps.tile([128, KT * OUT_F], F32, tag="ps")
                for j in range(NW - 1):
                    for kt in range(KT):
                        nc.tensor.matmul(
                            pm[:, kt * OUT_F : (kt + 1) * OUT_F],
                            cdt[:, j * 128 : (j + 1) * 128],
                            wallst[j][:, kt * OUT_F : (kt + 1) * OUT_F],
                            start=(j == 0),
                            stop=False,
                        )
                j = NW - 1
                for kt in range(KT):
                    nc.tensor.matmul(
                        pm[:, kt * OUT_F : (kt + 1) * OUT_F],
                        cdt[:, j * 128 : (j + 1) * 128],
                        wallst[j][:, kt * OUT_F : (kt + 1) * OUT_F],
                        start=False,
                        stop=True,
                    )
                    nc.scalar.copy(
                        wm[0][:, kt * OUT_F : (kt + 1) * OUT_F],
                        pm[:, kt * OUT_F : (kt + 1) * OUT_F],
                    )

                # ---- main GEMM ----
                for g in range(G_PER_CORE):
                    for ot in range(OT):
                        pt = ps.tile([128, TOK_PER_GROUP], F32, tag="ps")
                        for kt in range(KT):
                            lhsT = wm[g][
                                :, kt * OUT_F + ot * 128 : kt * OUT_F + (ot + 1) * 128
                            ]
                            for tci in range(TOK_PER_GROUP // 512):
                                nc.tensor.matmul(
                                    pt[:, tci * 512 : (tci + 1) * 512],
                                    lhsT,
                                    xslice(g, kt, tci),
                                    start=(kt == 0),
                                    stop=(kt == KT - 1),
                                )
                        oc = ocp.tile([128, TOK_PER_GROUP], F16, tag="oc")
                        nc.scalar.activation(
                            oc[:],
                            pt[:],
                            AF.Identity,
                            bias=mbT2[:, ot * G_PER_CORE + g : ot * G_PER_CORE + g + 1],
                            scale=1.0,
                        )
                        nc.scalar.dma_start(
                            outT[
                                ot * 128 : (ot + 1) * 128,
                                g * TOK_PER_GROUP : (g + 1) * TOK_PER_GROUP,
                            ],
                            oc[:],
                        )
    nc.finalize()
    return nc


def kernel(x, coefficients, weight_experts, bias_experts, weight_shared, bias_shared, sizes):
    x = np.asarray(x)
    coefficients = np.asarray(coefficients, dtype=np.float32)
    weight_experts = np.asarray(weight_experts, dtype=np.float32)
    bias_experts = np.asarray(bias_experts, dtype=np.float32)
    weight_shared = np.asarray(weight_shared, dtype=np.float32)
    bias_shared = np.asarray(bias_shared, dtype=np.float32)

    if "nc" not in _CACHE:
        _CACHE["nc"] = _build()
    nc = _CACHE["nc"]

    # ---- host-side layout prep ----
    x16 = x.astype(np.float16)
    # wall[p, j, kt*512+o] = W_j^T[kt*128+p, o]; j=0 shared, j=1+e expert e
    wall_np = np.empty((128, NW, KT * OUT_F), np.float16)
    for j in range(NW):
        W = weight_shared if j == 0 else weight_experts[j - 1]
        arr = W.T.reshape(KT, 128, OUT_F).transpose(1, 0, 2).reshape(128, KT * OUT_F)
        wall_np[:, j, :] = arr.astype(np.float16)
    ball_np = np.empty((NW, OUT_F), np.float32)
    ball_np[0] = bias_shared
    ball_np[1:] = bias_experts

    in_maps = []
    for c in range(N_CORES):
        gs = slice(c * G_PER_CORE, (c + 1) * G_PER_CORE)
        cg = coefficients[gs]  # [4, 8]
        cb_np = np.broadcast_to(
            cg.reshape(1, -1), (128, G_PER_CORE * N_EXPERTS)
        ).copy()
        cx_np = np.empty((NW, G_PER_CORE), np.float32)
        cx_np[0] = 1.0
        cx_np[1:] = cg.T
        cd_np = np.zeros((128, NW, 128), np.float16)
        idx = np.arange(128)
        cd_np[idx, 0, idx] = 1.0
        for e in range(N_EXPERTS):
            cd_np[idx, 1 + e, idx] = np.float16(cg[0, e])
        xT_np = np.ascontiguousarray(
            x16[c * TOK_PER_CORE : (c + 1) * TOK_PER_CORE].T
        )
        in_maps.append(
            {
                "xT": xT_np,
                "wall": wall_np,
                "cdiag": cd_np,
                "cb": cb_np,
                "cx": cx_np,
                "ball": ball_np,
            }
        )

    res = run_bass_kernel_spmd(nc, in_maps, core_ids=list(range(N_CORES)))
    out = np.empty((N_CORES * TOK_PER_CORE, OUT_F), np.float32)
    for c in range(N_CORES):
        out[c * TOK_PER_CORE : (c + 1) * TOK_PER_CORE] = (
            np.asarray(res.results[c]["outT"]).T.astype(np.float32)
        )
    return out


# revision 13
# speedup vs baseline: 1.0417x; 1.0417x over previous
"""MOLELinear (mixture-of-linear-experts) Trainium2 kernel.

Math (per group g): out_g = x_g @ (sum_e c[g,e] W_e + W_sh).T + (sum_e c[g,e] b_e + b_sh)

Sharding: data-parallel over the 32 groups -> 4 groups (8192 tokens) per core,
expert weights replicated. Host does layout-only prep (transpose / stacking /
fp16 rounding); all arithmetic of the reference runs on device.

Device plan per core (all fp16 data path, fp32 accumulation in PSUM):
  - DMA in: xT [512, 8192] fp16 (x shard transposed), wall [128, 9, 2048] fp16
    (shared + 8 experts, k-tile-major free layout), small coefficient/bias
    tensors, cdiag [128, 9, 128] (scaled identity matrices for group 0).
  - Group-0 weight mix on PE during the weight-DMA window:
    psum[:, kt] += diag(c_j) @ W_j[kt]  (keeps PE warm, fp32 accumulation).
  - Groups 1-3 weight mix on DVE: fp16 scalar_tensor_tensor FMA chains,
    FD=2048 per op (8 ops per group).
  - Mixed biases transposed on PE: mbT[ot][o,g] = sum_j ball[j,o] cx[j,g].
  - Main GEMM: stationary = mixed weight subtile [128k,128o], moving = xT
    slice [128k,512t]; psum [128 o, 2048 t] accumulates 4 k-tiles.
  - Drain on ScalarE via ACTIVATE(Identity, bias=mbT column): PSUM->SBUF fp16
    with the per-partition bias add fused in. DMA out on GpSimd (SWDGE) so
    output stores don't contend with the input DMA issue queue.
"""
import numpy as np

import concourse.bacc as bacc
import concourse.mybir as mybir
from concourse.alu_op_type import AluOpType
from concourse.tile import TileContext
from concourse.bass_utils import run_bass_kernel_spmd

N_CORES = 8
IN_F = 512
OUT_F = 512
N_EXPERTS = 8
N_GROUPS = 32
TOK_PER_GROUP = 2048
G_PER_CORE = N_GROUPS // N_CORES           # 4
TOK_PER_CORE = G_PER_CORE * TOK_PER_GROUP  # 8192
KT = IN_F // 128                           # 4 k-tiles
OT = OUT_F // 128                          # 4 out-feature tiles
NW = N_EXPERTS + 1                         # 9: shared weight first, then experts
F32 = mybir.dt.float32
F32R = mybir.dt.float32r
F16 = mybir.dt.float16
AF = mybir.ActivationFunctionType

_CACHE = {}


def _build():
    nc = bacc.Bacc(trn_type="TRN2")
    xT = nc.dram_tensor("xT", (IN_F, TOK_PER_CORE), F16, kind="ExternalInput")
    wall = nc.dram_tensor("wall", (128, NW, KT * OUT_F), F16, kind="ExternalInput")
    cdiag = nc.dram_tensor("cdiag", (128, NW, 128), F16, kind="ExternalInput")
    cb = nc.dram_tensor("cb", (128, G_PER_CORE * N_EXPERTS), F32, kind="ExternalInput")
    cx = nc.dram_tensor("cx", (NW, G_PER_CORE), F32R, kind="ExternalInput")
    ball = nc.dram_tensor("ball", (NW, OUT_F), F32R, kind="ExternalInput")
    outT = nc.dram_tensor("outT", (OUT_F, TOK_PER_CORE), F16, kind="ExternalOutput")

    with TileContext(nc) as tc:
        with (
            tc.tile_pool(name="smallp", bufs=1) as smallp,
            tc.tile_pool(name="wallp", bufs=1) as wallp,
            tc.tile_pool(name="wmp", bufs=1) as wmp,
            tc.tile_pool(name="xp", bufs=1) as xp,
            tc.tile_pool(name="ocp", bufs=4) as ocp,
        ):
            # ---- DMA issue order tuned so that: mixing chains start early
            # (cbt + expert 0 up front), wm0 (needs ALL experts) and xg0 land
            # at about the same time, later groups' x arrives last.
            cxt = smallp.tile([NW, G_PER_CORE], F32R, tag="cx")
            nc.sync.dma_start(cxt[:], cx[:])
            cbt = smallp.tile([128, G_PER_CORE * N_EXPERTS], F32, tag="cb")
            nc.sync.dma_start(cbt[:], cb[:])

            wallst = [
                wallp.tile([128, KT * OUT_F], F16, tag=f"wall{j}", name=f"wall{j}")
                for j in range(NW)
            ]
            nc.sync.dma_start(wallst[0][:], wall[:, 0, :])
            nc.sync.dma_start(wallst[1][:], wall[:, 1, :])

            cdt = smallp.tile([128, NW * 128], F16, tag="cdt")
            nc.sync.dma_start(cdt[:].rearrange("p (e m) -> p e m", e=NW), cdiag[:])
            ballt = smallp.tile([NW, OUT_F], F32R, tag="ball")
            nc.sync.dma_start(ballt[:], ball[:])

            for j in (2, 3, 4, 5, 6, 7):
                nc.sync.dma_start(wallst[j][:], wall[:, j, :])

            # group-0 x: per-k-tile tiles, interleaved with the last expert
            xg0t = [
                xp.tile([128, TOK_PER_GROUP], F16, tag=f"x0k{kt}", name=f"x0k{kt}")
                for kt in range(KT)
            ]
            nc.sync.dma_start(xg0t[0][:], xT[0:128, 0:TOK_PER_GROUP])
            nc.sync.dma_start(xg0t[1][:], xT[128:256, 0:TOK_PER_GROUP])
            nc.sync.dma_start(wallst[8][:], wall[:, 8, :])
            nc.sync.dma_start(xg0t[2][:], xT[256:384, 0:TOK_PER_GROUP])
            nc.sync.dma_start(xg0t[3][:], xT[384:512, 0:TOK_PER_GROUP])

            xg = [None]
            for g in range(1, G_PER_CORE):
                t = xp.tile([128, KT * TOK_PER_GROUP], F16, tag=f"x{g}", name=f"x{g}")
                nc.sync.dma_start(
                    t[:].rearrange("p (kt t) -> p kt t", kt=KT),
                    xT[:, g * TOK_PER_GROUP : (g + 1) * TOK_PER_GROUP].rearrange(
                        "(kt p) t -> p kt t", p=128
                    ),
                )
                xg.append(t)

            def xslice(g, kt, tci):
                if g == 0:
                    return xg0t[kt][:, tci * 512 : (tci + 1) * 512]
                return xg[g][
                    :,
                    kt * TOK_PER_GROUP + tci * 512 : kt * TOK_PER_GROUP + (tci + 1) * 512,
                ]

            wm = [
                wmp.tile([128, KT * OUT_F], F16, tag=f"wm{g}", name=f"wm{g}")
                for g in range(G_PER_CORE)
            ]

            with tc.tile_pool(name="ps", bufs=2, space="PSUM") as ps:
                # ---- mixed biases (one bank, freed early):
                # mbT2[o', ot*4+g] = sum_j ball[j, ot*128+o'] cx[j, g]
                pb = ps.tile([128, OT * G_PER_CORE], F32, tag="ps")
                for ot in range(OT):
                    nc.tensor.matmul(
                        pb[:, ot * G_PER_CORE : (ot + 1) * G_PER_CORE],
                        ballt[:, ot * 128 : (ot + 1) * 128],
                        cxt[:],
                        start=True,
                        stop=True,
                    )
                mbT2 = smallp.tile([128, OT * G_PER_CORE], F32, tag="mbT2")
                nc.scalar.copy(mbT2[:], pb[:])

                # ---- groups 1-3 weight mix on DVE, two-step per term:
                # tensor_scalar (4x mode) then tensor_tensor (2x mode)
                for g in (1, 2, 3):
                    for e in range(N_EXPERTS):
                        tmp = wmp.tile(
                            [128, KT * OUT_F], F16, tag="tmp", name="tmp", bufs=2
                        )
                        nc.vector.tensor_scalar(
                            tmp[:],
                            wallst[e + 1][:],
                            cbt[:, g * N_EXPERTS + e : g * N_EXPERTS + e + 1],
                            None,
                            AluOpType.mult,
                        )
                        nc.vector.tensor_tensor(
                            wm[g][:],
                            tmp[:],
                            wallst[0][:] if e == 0 else wm[g][:],
                            AluOpType.add,
                        )

                # ---- group-0 weight mix on PE (runs while weights stream in);
                # last expert's matmuls interleaved with per-k-tile casts so
                # wm[0] is ready ~1.5us after the final weight slice lands
                pm = ps.tile([128, KT * OUT_F], F32, tag="ps")
                for j in range(NW - 1):
                    for kt in range(KT):
                        nc.tensor.matmul(
                            pm[:, kt * OUT_F : (kt + 1) * OUT_F],
                            cdt[:, j * 128 : (j + 1) * 128],
                            wallst[j][:, kt * OUT_F : (kt + 1) * OUT_F],
                            start=(j == 0),
                            stop=False,
                        )
                j = NW - 1
                for kt in range(KT):
                    nc.tensor.matmul(
                        pm[:, kt * OUT_F : (kt + 1) * OUT_F],
                        cdt[:, j * 128 : (j + 1) * 128],
                        wallst[j][:, kt * OUT_F : (kt + 1) * OUT_F],
                        start=False,
                        stop=True,
                    )
                    nc.scalar.copy(
                        wm[0][:, kt * OUT_F : (kt + 1) * OUT_F],
                        pm[:, kt * OUT_F : (kt + 1) * OUT_F],
                    )

                # ---- main GEMM ----
                for g in range(G_PER_CORE):
                    for ot in range(OT):
                        pt = ps.tile([128, TOK_PER_GROUP], F32, tag="ps")
                        for kt in range(KT):
                            lhsT = wm[g][
                                :, kt * OUT_F + ot * 128 : kt * OUT_F + (ot + 1) * 128
                            ]
                            for tci in range(TOK_PER_GROUP // 512):
                                nc.tensor.matmul(
                                    pt[:, tci * 512 : (tci + 1) * 512],
                                    lhsT,
                                    xslice(g, kt, tci),
                                    start=(kt == 0),
                                    stop=(kt == KT - 1),
                                )
                        oc = ocp.tile([128, TOK_PER_GROUP], F16, tag="oc")
                        bias_ap = mbT2[:, ot * G_PER_CORE + g : ot * G_PER_CORE + g + 1]
                        last = g == G_PER_CORE - 1 and ot == OT - 1
                        halves = 2 if last else 1
                        hw = TOK_PER_GROUP // halves
                        for h in range(halves):
                            nc.scalar.activation(
                                oc[:, h * hw : (h + 1) * hw],
                                pt[:, h * hw : (h + 1) * hw],
                                AF.Identity,
                                bias=bias_ap,
                                scale=1.0,
                            )
                            nc.scalar.dma_start(
                                outT[
                                    ot * 128 : (ot + 1) * 128,
                                    g * TOK_PER_GROUP + h * hw : g * TOK_PER_GROUP
                                    + (h + 1) * hw,
                                ],
                                oc[:, h * hw : (h + 1) * hw],
                            )
    nc.finalize()
    return nc


def kernel(x, coefficients, weight_experts, bias_experts, weight_shared, bias_shared, sizes):
    x = np.asarray(x)
    coefficients = np.asarray(coefficients, dtype=np.float32)
    weight_experts = np.asarray(weight_experts, dtype=np.float32)
    bias_experts = np.asarray(bias_experts, dtype=np.float32)
    weight_shared = np.asarray(weight_shared, dtype=np.float32)
    bias_shared = np.asarray(bias_shared, dtype=np.float32)

    if "nc" not in _CACHE:
        _CACHE["nc"] = _build()
    nc = _CACHE["nc"]

    # ---- host-side layout prep ----
    x16 = x.astype(np.float16)
    # wall[p, j, kt*512+o] = W_j^T[kt*128+p, o]; j=0 shared, j=1+e expert e
    wall_np = np.empty((128, NW, KT * OUT_F), np.float16)
    for j in range(NW):
        W = weight_shared if j == 0 else weight_experts[j - 1]
        arr = W.T.reshape(KT, 128, OUT_F).transpose(1, 0, 2).reshape(128, KT * OUT_F)
        wall_np[:, j, :] = arr.astype(np.float16)
    ball_np = np.empty((NW, OUT_F), np.float32)
    ball_np[0] = bias_shared
    ball_np[1:] = bias_experts

    in_maps = []
    for c in range(N_CORES):
        gs = slice(c * G_PER_CORE, (c + 1) * G_PER_CORE)
        cg = coefficients[gs]  # [4, 8]
        cb_np = np.broadcast_to(
            cg.reshape(1, -1), (128, G_PER_CORE * N_EXPERTS)
        ).copy()
        cx_np = np.empty((NW, G_PER_CORE), np.float32)
        cx_np[0] = 1.0
        cx_np[1:] = cg.T
        cd_np = np.zeros((128, NW, 128), np.float16)
        idx = np.arange(128)
        cd_np[idx, 0, idx] = 1.0
        for e in range(N_EXPERTS):
            cd_np[idx, 1 + e, idx] = np.float16(cg[0, e])
        xT_np = np.ascontiguousarray(
            x16[c * TOK_PER_CORE : (c + 1) * TOK_PER_CORE].T
        )
        in_maps.append(
            {
                "xT": xT_np,
                "wall": wall_np,
                "cdiag": cd_np,
                "cb": cb_np,
                "cx": cx_np,
                "ball": ball_np,
            }
        )

    res = run_bass_kernel_spmd(nc, in_maps, core_ids=list(range(N_CORES)))
    out = np.empty((N_CORES * TOK_PER_CORE, OUT_F), np.float32)
    for c in range(N_CORES):
        out[c * TOK_PER_CORE : (c + 1) * TOK_PER_CORE] = (
            np.asarray(res.results[c]["outT"]).T.astype(np.float32)
        )
    return out
